# revision 9
# baseline (speedup 1.0000x reference)
"""Trainium2 Bass kernel for nn_Net_27358941676189 (sparse_attention).

Key structure exploited: the reference indexes the embedding table with the
*binarized* input (rows 0/1 only), so every q / k row takes one of two values.
The [B,G,G] attention-score tensor is therefore

    attn_weights[b,g,h] = bias[g,h] + A[b,g] + Bc[b,g] * xh[b,h]

(a shared full-rank bias plus a per-batch rank-2 term), where
    A  = c00 + xg*(c10-c00),       Bc = (c01-c00) + xg*((c11-c10)-(c01-c00)),
    cij = (Wq @ e_i) . (Wk @ e_j),  xg/xh = (datax != 0).

Row-softmax + dot with the binary value vector collapses algebraically:
    attn[b,g] = eB*S1 / (S0m1 + eB*S1),   eB = exp(Bc[b,g]),
    S1   = sum_h expb[g,h] * xh[b,h],
    S0m1 = sum_h expb[g,h] * (1 - xh[b,h]),   expb = exp(bias - 20).

Sharding: the attention row dim G is split across the 8 cores (256 rows
each, all batches).  Each core computes bias for its rows from the five raw
[256,G] inputs, emits its [B,256,G] slice of attn_weights_origin, computes
S1/S0m1 (PE transpose + matmul), the closed-form attn rows, and a partial
fc1 product over its 256 columns of fc1_w.  The host sums the 8 fc1
partials and applies the tiny fc1_b / fcCox tail.
"""

import os
import sys

import numpy as np

G = 2048
E = 512
B = 16
NCORES = 8
R = G // NCORES            # 256 rows per core
TPC = R // 128             # 2 row-tiles of 128 per core
HCH = G // 128             # 16 column chunks for the S1 matmuls
EXP_SHIFT = -20.0          # exp(bias + EXP_SHIFT): range headroom, cancels in the ratio


def _ensure_path():
    p = "/opt/trn_rl_repo"
    if os.path.isdir(p) and p not in sys.path:
        sys.path.insert(0, p)


def _leaky(x):
    return np.where(x > 0, x, 0.01 * x)


def build_nc(repeat=1):
    """Build the per-core Bass/Tile program (same program on all 8 cores).

    repeat>1 wraps the whole body in a hardware loop — used only by the
    timing harness (bench.py) to amortize dispatch overhead.
    """
    _ensure_path()
    import concourse.bacc as bacc
    import concourse.tile as tile
    from concourse import mybir
    from concourse.masks import make_identity
    from contextlib import ExitStack

    f32 = mybir.dt.float32
    AF = mybir.ActivationFunctionType
    OP = mybir.AluOpType

    # Bacc (not raw Bass): its codegen legalizes multi-semaphore waits that
    # walrus's per-instruction sync-wait slots can't encode.
    nc = bacc.Bacc("TRN2")

    # --- DRAM I/O (per-core shapes; data differs per core via in_maps) ---
    kk1 = nc.dram_tensor("kk1", [TPC, 128, G], f32, kind="ExternalInput")
    ksp = nc.dram_tensor("ksp", [TPC, 128, G], f32, kind="ExternalInput")
    kk2 = nc.dram_tensor("kk2", [TPC, 128, G], f32, kind="ExternalInput")
    kcen = nc.dram_tensor("kcen", [TPC, 128, G], f32, kind="ExternalInput")
    kpad = nc.dram_tensor("kpad", [TPC, 128, G], f32, kind="ExternalInput")  # pre-scaled by lrelu(k3)
    rhs2 = nc.dram_tensor("rhs2", [B, 2, G], f32, kind="ExternalInput")      # [ones; xh_b]
    ab = nc.dram_tensor("ab", [2, B * R], f32, kind="ExternalInput")         # rows A, Bc (b-major)
    xbt2 = nc.dram_tensor("xbt2", [128, HCH, 32], f32, kind="ExternalInput") # [xh.T | (1-xh).T] chunks
    ebt = nc.dram_tensor("ebt", [128, TPC, B], f32, kind="ExternalInput")    # exp(Bc) slice, [p,t,b]
    xgt = nc.dram_tensor("xgt", [128, TPC, B], f32, kind="ExternalInput")    # xg slice, [p,t,b]
    fc1t = nc.dram_tensor("fc1t", [TPC, 128, G], f32, kind="ExternalInput")  # fc1_w[:, slice].T

    wout = nc.dram_tensor("wout", [B, R, G], f32, kind="ExternalOutput")
    fc1p = nc.dram_tensor("fc1p", [B, G], f32, kind="ExternalOutput")

    with tile.TileContext(nc) as tc, ExitStack() as ctx:
        const = ctx.enter_context(tc.tile_pool(name="const", bufs=1))
        kin = ctx.enter_context(tc.tile_pool(name="kin", bufs=6))
        biasp = ctx.enter_context(tc.tile_pool(name="biasp", bufs=2))
        expp = ctx.enter_context(tc.tile_pool(name="expp", bufs=2))
        expTp = ctx.enter_context(tc.tile_pool(name="expTp", bufs=3))
        smalls = ctx.enter_context(tc.tile_pool(name="smalls", bufs=8))
        afcp = ctx.enter_context(tc.tile_pool(name="afcp", bufs=2))
        rhs2p = ctx.enter_context(tc.tile_pool(name="rhs2p", bufs=2))
        fc1tp = ctx.enter_context(tc.tile_pool(name="fc1tp", bufs=2))
        wsbp = ctx.enter_context(tc.tile_pool(name="wsbp", bufs=3))
        fc1sbp = ctx.enter_context(tc.tile_pool(name="fc1sbp", bufs=1))

        ident = const.tile([128, 128], f32)
        make_identity(nc, ident[:])
        shiftc = const.tile([128, 1], f32)
        nc.vector.memset(shiftc, EXP_SHIFT)
        xbt2s = const.tile([128, HCH, 32], f32)
        nc.sync.dma_start(out=xbt2s, in_=xbt2[:])
        ebts = const.tile([128, TPC, B], f32)
        nc.sync.dma_start(out=ebts, in_=ebt[:])
        xgts = const.tile([128, TPC, B], f32)
        nc.sync.dma_start(out=xgts, in_=xgt[:])
        absb = const.tile([2, B * R], f32)
        nc.sync.dma_start(out=absb, in_=ab[:])

        fc1t_tiles = []
        for t in range(TPC):
            ft = fc1tp.tile([128, G], f32, tag="fc1t")
            nc.sync.dma_start(out=ft, in_=fc1t[t])
            fc1t_tiles.append(ft)

        def _kernel_body():
            bias_tiles = []
            afc_tiles = []

            # PSUM budget: tp(1) + s1(1) + w(3 x 2 banks) = 8 banks.
            with (
                tc.tile_pool(name="tpps", bufs=1, space="PSUM") as tpps,
                tc.tile_pool(name="s1ps", bufs=1, space="PSUM") as s1ps,
                tc.tile_pool(name="wps", bufs=3, space="PSUM") as wps,
            ):
                # ---------- phase 1: bias, exp, S1/S0m1, attn rows ----------
                for t in range(TPC):
                    k1t = kin.tile([128, G], f32, tag="kin")
                    nc.sync.dma_start(out=k1t, in_=kk1[t])
                    spt = kin.tile([128, G], f32, tag="kin")
                    nc.sync.dma_start(out=spt, in_=ksp[t])
                    k2t = kin.tile([128, G], f32, tag="kin")
                    nc.sync.dma_start(out=k2t, in_=kk2[t])
                    cent = kin.tile([128, G], f32, tag="kin")
                    nc.sync.dma_start(out=cent, in_=kcen[t])
                    padt = kin.tile([128, G], f32, tag="kin")
                    nc.sync.dma_start(out=padt, in_=kpad[t])

                    # Single-engine (DVE) chain: same-engine program order avoids
                    # cross-engine semaphore fan-in (HW limits sync waits/inst).
                    # leaky_relu(x) = max(0.01*x, x), in place.
                    bias_t = biasp.tile([128, G], f32, tag="bias")
                    nc.vector.scalar_tensor_tensor(
                        out=k1t, in0=k1t, scalar=0.01, in1=k1t, op0=OP.mult, op1=OP.max
                    )
                    nc.vector.tensor_tensor(out=bias_t, in0=k1t, in1=spt, op=OP.mult)
                    nc.vector.scalar_tensor_tensor(
                        out=k2t, in0=k2t, scalar=0.01, in1=k2t, op0=OP.mult, op1=OP.max
                    )
                    nc.vector.tensor_tensor(out=k2t, in0=k2t, in1=cent, op=OP.mult)
                    nc.vector.tensor_tensor(out=bias_t, in0=bias_t, in1=k2t, op=OP.add)
                    nc.vector.tensor_tensor(out=bias_t, in0=bias_t, in1=padt, op=OP.add)
                    bias_tiles.append(bias_t)

                    exp_t = expp.tile([128, G], f32, tag="exp")
                    nc.scalar.activation(out=exp_t, in_=bias_t, func=AF.Exp, bias=shiftc[:])

                    s1p = s1ps.tile([128, 32], f32, tag="s1")
                    for c in range(HCH):
                        psT = tpps.tile([128, 128], f32, tag="tp")
                        nc.tensor.transpose(psT, exp_t[:, c * 128:(c + 1) * 128], ident)
                        expTc = expTp.tile([128, 128], f32, tag="expT")
                        nc.scalar.copy(out=expTc, in_=psT)
                        nc.tensor.matmul(
                            s1p,
                            lhsT=expTc,
                            rhs=xbt2s[:, c, :],
                            start=(c == 0),
                            stop=(c == HCH - 1),
                        )

                    s1sb = smalls.tile([128, 32], f32, tag="s1sb")
                    nc.scalar.copy(out=s1sb, in_=s1p)
                    t1 = smalls.tile([128, B], f32, tag="t1")
                    nc.vector.tensor_tensor(out=t1, in0=s1sb[:, 0:B], in1=ebts[:, t, :], op=OP.mult)
                    den = smalls.tile([128, B], f32, tag="den")
                    nc.vector.tensor_tensor(out=den, in0=t1, in1=s1sb[:, B:2 * B], op=OP.add)
                    rec = smalls.tile([128, B], f32, tag="rec")
                    nc.vector.reciprocal(out=rec, in_=den)
                    afc = afcp.tile([128, B], f32, tag="afc")
                    nc.vector.tensor_tensor(out=afc, in0=t1, in1=rec, op=OP.mult)
                    nc.vector.tensor_tensor(out=afc, in0=afc, in1=xgts[:, t, :], op=OP.add)
                    afc_tiles.append(afc)

                # ---------- phase 2: emit attn_weights_origin slices ----------
                for b in range(B):
                    r2 = rhs2p.tile([2, G], f32, tag="r2")
                    nc.sync.dma_start(out=r2, in_=rhs2[b])
                    for t in range(TPC):
                        lhs = absb[:, b * R + t * 128: b * R + (t + 1) * 128]
                        wsb = wsbp.tile([128, G], f32, tag="wsb")
                        for h in range(2):  # halves of 1024 cols -> 2-bank psum tiles
                            wp = wps.tile([128, 1024], f32, tag="wp")
                            for n in range(2):  # 512-col matmul blocks (f32 moving max)
                                lo = h * 1024 + n * 512
                                nc.tensor.matmul(
                                    wp[:, n * 512:(n + 1) * 512],
                                    lhsT=lhs,
                                    rhs=r2[:, lo:lo + 512],
                                    start=True,
                                    stop=True,
                                )
                            nc.vector.tensor_tensor(
                                out=wsb[:, h * 1024:(h + 1) * 1024],
                                in0=wp,
                                in1=bias_tiles[t][:, h * 1024:(h + 1) * 1024],
                                op=OP.add,
                            )
                        nc.sync.dma_start(out=wout[b, t * 128:(t + 1) * 128, :], in_=wsb)

            # ---------- fc1 partial: (attn + xg) @ fc1_w[:, slice].T ----------
            with tc.tile_pool(name="fc1ps", bufs=2, space="PSUM") as fc1ps:
                fc1sb = fc1sbp.tile([B, G], f32, tag="fc1sb")
                for c2 in range(2):
                    fp = fc1ps.tile([B, 1024], f32, tag="fp")
                    for t in range(TPC):
                        for n in range(2):
                            lo = c2 * 1024 + n * 512
                            nc.tensor.matmul(
                                fp[:, n * 512:(n + 1) * 512],
                                lhsT=afc_tiles[t],
                                rhs=fc1t_tiles[t][:, lo:lo + 512],
                                start=(t == 0),
                                stop=(t == TPC - 1),
                            )
                    nc.scalar.copy(out=fc1sb[:, c2 * 1024:(c2 + 1) * 1024], in_=fp)
                nc.sync.dma_start(out=fc1p[:], in_=fc1sb)

        if repeat == 1:
            _kernel_body()
        else:
            with tc.For_i(0, repeat, 1):
                _kernel_body()

    nc.finalize()  # Bacc pass pipeline: event-sem legalization, reg alloc, ISA codegen
    return nc


def host_prep(inputs):
    """Host-side scalar/layout prep. Returns per-core in_maps."""
    f32 = np.float32
    f64 = np.float64
    datax = np.asarray(inputs["datax"])
    embMat = np.asarray(inputs["embMat"], f32)
    Wq = np.asarray(inputs["Wq"], f32)
    Wk = np.asarray(inputs["Wk"], f32)
    k1 = np.ascontiguousarray(np.asarray(inputs["k1"], f32))
    k2 = np.ascontiguousarray(np.asarray(inputs["k2"], f32))
    k3 = np.asarray(inputs["k3"], f32)
    sp = np.ascontiguousarray(np.asarray(inputs["shortestPath"], f32))
    cen = np.ascontiguousarray(np.asarray(inputs["centralityMat"], f32))
    pad = np.asarray(inputs["padding"], f32)
    fc1_w = np.asarray(inputs["fc1_w"], f32)

    xbin = datax != 0
    x0 = xbin.astype(f32)                          # [B,G]

    e0 = embMat[0].astype(f64)
    e1 = embMat[1].astype(f64)
    q0 = Wq.astype(f64) @ e0
    q1 = Wq.astype(f64) @ e1
    kv0 = Wk.astype(f64) @ e0
    kv1 = Wk.astype(f64) @ e1
    c00 = q0 @ kv0
    c01 = q0 @ kv1
    c10 = q1 @ kv0
    c11 = q1 @ kv1

    xg = x0.astype(f64)
    A = c00 + xg * (c10 - c00)                     # [B,G]
    Bc = (c01 - c00) + xg * ((c11 - c10) - (c01 - c00))
    eB = np.exp(Bc)

    k3lr = float(_leaky(k3)[0])
    padk = np.ascontiguousarray(pad * k3lr).astype(f32)

    rhs2 = np.empty((B, 2, G), f32)
    rhs2[:, 0, :] = 1.0
    rhs2[:, 1, :] = x0

    xbt2 = np.empty((G, 32), f32)
    xbt2[:, :B] = x0.T
    xbt2[:, B:] = 1.0 - x0.T
    xbt2p = np.ascontiguousarray(xbt2.reshape(HCH, 128, 32).transpose(1, 0, 2))

    fc1wT = np.ascontiguousarray(fc1_w.T)          # [g, j]

    def tslice(arr_bg, sl):                        # [B,Rslice] -> [128, TPC, B]
        return np.ascontiguousarray(
            arr_bg[:, sl].T.reshape(TPC, 128, B).transpose(1, 0, 2)
        ).astype(f32)

    in_maps = []
    for c in range(NCORES):
        sl = slice(c * R, (c + 1) * R)
        in_maps.append({
            "kk1": k1[sl].reshape(TPC, 128, G),
            "ksp": sp[sl].reshape(TPC, 128, G),
            "kk2": k2[sl].reshape(TPC, 128, G),
            "kcen": cen[sl].reshape(TPC, 128, G),
            "kpad": padk[sl].reshape(TPC, 128, G),
            "rhs2": rhs2,
            "ab": np.ascontiguousarray(
                np.stack([A[:, sl].reshape(-1), Bc[:, sl].reshape(-1)])
            ).astype(f32),
            "xbt2": xbt2p,
            "ebt": tslice(eB, sl),
            "xgt": tslice(xg, sl),
            "fc1t": fc1wT[sl].reshape(TPC, 128, G),
        })
    return in_maps


def kernel(**inputs):
    _ensure_path()
    from concourse.bass_utils import run_bass_kernel_spmd

    in_maps = host_prep(inputs)
    nc = build_nc()

    res = run_bass_kernel_spmd(nc, in_maps, core_ids=list(range(NCORES)))
    kernel.last_results = res
    kernel.last_exec_time_ns = res.exec_time_ns

    f32 = np.float32
    attn_w = np.empty((B, G, G), f32)
    fc1sum = np.zeros((B, G), np.float64)
    for c in range(NCORES):
        r = res.results[c]
        attn_w[:, c * R:(c + 1) * R, :] = r["wout"]
        fc1sum += r["fc1p"]

    fc1_b = np.asarray(inputs["fc1_b"], f32)
    fcCox_w = np.asarray(inputs["fcCox_w"], f32)
    y = fc1sum + fc1_b.astype(np.float64)
    out = (y @ fcCox_w[0].astype(np.float64)).astype(f32)
    return out, attn_w


# revision 20
# speedup vs baseline: 1.0444x; 1.0444x over previous
"""Trainium2 Bass kernel for nn_Net_27358941676189 (sparse_attention).

Key structure exploited: the reference indexes the embedding table with the
*binarized* input (rows 0/1 only), so every q / k row takes one of two values.
The [B,G,G] attention-score tensor is therefore

    attn_weights[b,g,h] = bias[g,h] + A[b,g] + Bc[b,g] * xh[b,h]

(a shared full-rank bias plus a per-batch rank-2 term), where
    A  = c00 + xg*(c10-c00),       Bc = (c01-c00) + xg*((c11-c10)-(c01-c00)),
    cij = (Wq @ e_i) . (Wk @ e_j),  xg/xh = (datax != 0).

Row-softmax + dot with the binary value vector collapses algebraically:
    attn[b,g] = eB*S1 / (S0m1 + eB*S1),   eB = exp(Bc[b,g]),
    S1   = sum_h expb[g,h] * xh[b,h],
    S0m1 = sum_h expb[g,h] * (1 - xh[b,h]),   expb = exp(bias - 20).

Sharding: the attention row dim G is split across the 8 cores (256 rows
each, all batches).  Each core computes bias for its rows from the five raw
[256,G] inputs, emits its [B,256,G] slice of attn_weights_origin, computes
S1/S0m1 (PE transpose + matmul), the closed-form attn rows, and a partial
fc1 product over its 256 columns of fc1_w.  The host sums the 8 fc1
partials and applies the tiny fc1_b / fcCox tail.
"""

import os
import sys

import numpy as np

G = 2048
E = 512
B = 16
NCORES = 8
R = G // NCORES            # 256 rows per core
TPC = R // 128             # 2 row-tiles of 128 per core
HCH = G // 128             # 16 column chunks for the S1 matmuls
EXP_SHIFT = -20.0          # exp(bias + EXP_SHIFT): range headroom, cancels in the ratio


def _ensure_path():
    p = "/opt/trn_rl_repo"
    if os.path.isdir(p) and p not in sys.path:
        sys.path.insert(0, p)


def _leaky(x):
    return np.where(x > 0, x, 0.01 * x)


def build_nc(repeat=1, packed_rhs2=True):
    """Build the per-core Bass/Tile program (same program on all 8 cores).

    repeat>1 wraps the whole body in a hardware loop — used only by the
    timing harness (bench.py) to amortize dispatch overhead.
    packed_rhs2: keep all 16 [ones; xh_b] rows resident in one [32,G] tile
    (matmul operands at partition offsets 2b) instead of per-batch DMAs.
    """
    _ensure_path()
    import concourse.bacc as bacc
    import concourse.tile as tile
    from concourse import mybir
    from concourse.masks import make_identity
    from contextlib import ExitStack

    f32 = mybir.dt.float32
    AF = mybir.ActivationFunctionType
    OP = mybir.AluOpType

    # Bacc (not raw Bass): its codegen legalizes multi-semaphore waits that
    # walrus's per-instruction sync-wait slots can't encode.
    nc = bacc.Bacc("TRN2")

    # --- DRAM I/O (per-core shapes; data differs per core via in_maps) ---
    kk1 = nc.dram_tensor("kk1", [TPC, 128, G], f32, kind="ExternalInput")
    ksp = nc.dram_tensor("ksp", [TPC, 128, G], f32, kind="ExternalInput")
    kk2 = nc.dram_tensor("kk2", [TPC, 128, G], f32, kind="ExternalInput")
    kcen = nc.dram_tensor("kcen", [TPC, 128, G], f32, kind="ExternalInput")
    kpad = nc.dram_tensor("kpad", [TPC, 128, G], f32, kind="ExternalInput")  # pre-scaled by lrelu(k3)
    rhs2 = nc.dram_tensor("rhs2", [B, 2, G], f32, kind="ExternalInput")      # [ones; xh_b]
    # Block-diagonal A/Bc: column block b has rows 2b/2b+1 = A/Bc, rest 0.
    # Lets the packed [32,G] rhs2 tile be a matmul operand at partition 0
    # (PE requires operand base partition in {0,32,64}), with K=32.
    ab = nc.dram_tensor("ab", [2 * B, B * R], f32, kind="ExternalInput")
    xbt2 = nc.dram_tensor("xbt2", [128, HCH, 32], f32, kind="ExternalInput") # [xh.T | (1-xh).T] chunks
    ebt = nc.dram_tensor("ebt", [128, TPC, B], f32, kind="ExternalInput")    # exp(Bc) slice, [p,t,b]
    xgt = nc.dram_tensor("xgt", [128, TPC, B], f32, kind="ExternalInput")    # xg slice, [p,t,b]
    fc1t = nc.dram_tensor("fc1t", [TPC, 128, G], f32, kind="ExternalInput")  # fc1_w[:, slice].T

    wout = nc.dram_tensor("wout", [B, R, G], f32, kind="ExternalOutput")
    fc1p = nc.dram_tensor("fc1p", [B, G], f32, kind="ExternalOutput")

    with tile.TileContext(nc) as tc, ExitStack() as ctx:
        const = ctx.enter_context(tc.tile_pool(name="const", bufs=1))
        kin = ctx.enter_context(tc.tile_pool(name="kin", bufs=6))
        biasp = ctx.enter_context(tc.tile_pool(name="biasp", bufs=2))
        expp = ctx.enter_context(tc.tile_pool(name="expp", bufs=2))
        expTp = ctx.enter_context(tc.tile_pool(name="expTp", bufs=3))
        smalls = ctx.enter_context(tc.tile_pool(name="smalls", bufs=8))
        afcp = ctx.enter_context(tc.tile_pool(name="afcp", bufs=2))
        fc1tp = ctx.enter_context(tc.tile_pool(name="fc1tp", bufs=2))
        wsbp = ctx.enter_context(tc.tile_pool(name="wsbp", bufs=3))
        fc1sbp = ctx.enter_context(tc.tile_pool(name="fc1sbp", bufs=1))

        ident = const.tile([128, 128], f32)
        make_identity(nc, ident[:])
        shiftc = const.tile([128, 1], f32)
        nc.vector.memset(shiftc, EXP_SHIFT)
        xbt2s = const.tile([128, HCH, 32], f32)
        nc.sync.dma_start(out=xbt2s, in_=xbt2[:])
        ebts = const.tile([128, TPC, B], f32)
        nc.sync.dma_start(out=ebts, in_=ebt[:])
        xgts = const.tile([128, TPC, B], f32)
        nc.sync.dma_start(out=xgts, in_=xgt[:])
        absb = const.tile([2 * B, B * R], f32)
        nc.sync.dma_start(out=absb, in_=ab[:])
        rhs2s = const.tile([2 * B, G], f32)
        nc.sync.dma_start(out=rhs2s, in_=rhs2[:])

        fc1t_tiles = []
        for t in range(TPC):
            ft = fc1tp.tile([128, G], f32, tag="fc1t")
            nc.gpsimd.dma_start(out=ft, in_=fc1t[t])
            fc1t_tiles.append(ft)

        def _kernel_body():
            bias_tiles = []
            afc_tiles = []

            # PSUM budget: tp(1) + s1(1) + w(3 x 2 banks) = 8 banks.
            with (
                tc.tile_pool(name="tpps", bufs=1, space="PSUM") as tpps,
                tc.tile_pool(name="s1ps", bufs=1, space="PSUM") as s1ps,
                tc.tile_pool(name="wps", bufs=3, space="PSUM") as wps,
            ):
                # ---------- phase 1: bias, exp, S1/S0m1, attn rows ----------
                # Spread the 10 big input loads across all three DMA rings
                # (SWDGE + both HWDGE) so no single ring serializes them.
                ld = [nc.gpsimd, nc.sync, nc.scalar]
                for t in range(TPC):
                    k1t = kin.tile([128, G], f32, tag="kin")
                    ld[(5 * t + 0) % 3].dma_start(out=k1t, in_=kk1[t])
                    spt = kin.tile([128, G], f32, tag="kin")
                    ld[(5 * t + 1) % 3].dma_start(out=spt, in_=ksp[t])
                    k2t = kin.tile([128, G], f32, tag="kin")
                    ld[(5 * t + 2) % 3].dma_start(out=k2t, in_=kk2[t])
                    cent = kin.tile([128, G], f32, tag="kin")
                    ld[(5 * t + 3) % 3].dma_start(out=cent, in_=kcen[t])
                    padt = kin.tile([128, G], f32, tag="kin")
                    ld[(5 * t + 4) % 3].dma_start(out=padt, in_=kpad[t])

                    # Single-engine (DVE) chain: same-engine program order avoids
                    # cross-engine semaphore fan-in (HW limits sync waits/inst).
                    # leaky_relu(x) = max(0.01*x, x), in place.
                    bias_t = biasp.tile([128, G], f32, tag="bias")
                    nc.vector.scalar_tensor_tensor(
                        out=k1t, in0=k1t, scalar=0.01, in1=k1t, op0=OP.mult, op1=OP.max
                    )
                    nc.vector.tensor_tensor(out=bias_t, in0=k1t, in1=spt, op=OP.mult)
                    nc.vector.scalar_tensor_tensor(
                        out=k2t, in0=k2t, scalar=0.01, in1=k2t, op0=OP.mult, op1=OP.max
                    )
                    nc.vector.tensor_tensor(out=k2t, in0=k2t, in1=cent, op=OP.mult)
                    nc.vector.tensor_tensor(out=bias_t, in0=bias_t, in1=k2t, op=OP.add)
                    nc.vector.tensor_tensor(out=bias_t, in0=bias_t, in1=padt, op=OP.add)
                    bias_tiles.append(bias_t)

                    exp_t = expp.tile([128, G], f32, tag="exp")
                    nc.scalar.activation(out=exp_t, in_=bias_t, func=AF.Exp, bias=shiftc[:])

                    s1p = s1ps.tile([128, 32], f32, tag="s1")
                    for c in range(HCH):
                        psT = tpps.tile([128, 128], f32, tag="tp")
                        nc.tensor.transpose(psT, exp_t[:, c * 128:(c + 1) * 128], ident)
                        expTc = expTp.tile([128, 128], f32, tag="expT")
                        nc.scalar.copy(out=expTc, in_=psT)
                        nc.tensor.matmul(
                            s1p,
                            lhsT=expTc,
                            rhs=xbt2s[:, c, :],
                            start=(c == 0),
                            stop=(c == HCH - 1),
                        )

                    s1sb = smalls.tile([128, 32], f32, tag="s1sb")
                    nc.scalar.copy(out=s1sb, in_=s1p)
                    t1 = smalls.tile([128, B], f32, tag="t1")
                    nc.vector.tensor_tensor(out=t1, in0=s1sb[:, 0:B], in1=ebts[:, t, :], op=OP.mult)
                    den = smalls.tile([128, B], f32, tag="den")
                    nc.vector.tensor_tensor(out=den, in0=t1, in1=s1sb[:, B:2 * B], op=OP.add)
                    rec = smalls.tile([128, B], f32, tag="rec")
                    nc.vector.reciprocal(out=rec, in_=den)
                    afc = afcp.tile([128, B], f32, tag="afc")
                    nc.vector.tensor_tensor(out=afc, in0=t1, in1=rec, op=OP.mult)
                    nc.vector.tensor_tensor(out=afc, in0=afc, in1=xgts[:, t, :], op=OP.add)
                    afc_tiles.append(afc)

                # ---------- phase 2: emit attn_weights_origin slices ----------
                # t-major so the t=0 output stream starts while t=1's bias is
                # still being computed; alternate output DMAs across the two
                # HWDGE rings (SP / ACT).
                for t in range(TPC):
                    for b in range(B):
                        r2 = rhs2s[:, :]
                        lhs = absb[:, b * R + t * 128: b * R + (t + 1) * 128]
                        wsb = wsbp.tile([128, G], f32, tag="wsb")
                        for h in range(2):  # halves of 1024 cols -> 2-bank psum tiles
                            wp = wps.tile([128, 1024], f32, tag="wp")
                            for n in range(2):  # 512-col matmul blocks (f32 moving max)
                                lo = h * 1024 + n * 512
                                nc.tensor.matmul(
                                    wp[:, n * 512:(n + 1) * 512],
                                    lhsT=lhs,
                                    rhs=r2[:, lo:lo + 512],
                                    start=True,
                                    stop=True,
                                )
                            nc.vector.tensor_tensor(
                                out=wsb[:, h * 1024:(h + 1) * 1024],
                                in0=wp,
                                in1=bias_tiles[t][:, h * 1024:(h + 1) * 1024],
                                op=OP.add,
                            )
                        eng = nc.sync if b % 2 == 0 else nc.scalar
                        eng.dma_start(out=wout[b, t * 128:(t + 1) * 128, :], in_=wsb)

            # ---------- fc1 partial: (attn + xg) @ fc1_w[:, slice].T ----------
            with tc.tile_pool(name="fc1ps", bufs=2, space="PSUM") as fc1ps:
                fc1sb = fc1sbp.tile([B, G], f32, tag="fc1sb")
                for c2 in range(2):
                    fp = fc1ps.tile([B, 1024], f32, tag="fp")
                    for t in range(TPC):
                        for n in range(2):
                            lo = c2 * 1024 + n * 512
                            nc.tensor.matmul(
                                fp[:, n * 512:(n + 1) * 512],
                                lhsT=afc_tiles[t],
                                rhs=fc1t_tiles[t][:, lo:lo + 512],
                                start=(t == 0),
                                stop=(t == TPC - 1),
                            )
                    nc.scalar.copy(out=fc1sb[:, c2 * 1024:(c2 + 1) * 1024], in_=fp)
                nc.sync.dma_start(out=fc1p[:], in_=fc1sb)

        if repeat == 1:
            _kernel_body()
        else:
            with tc.For_i(0, repeat, 1):
                _kernel_body()

    nc.finalize()  # Bacc pass pipeline: event-sem legalization, reg alloc, ISA codegen
    return nc


def host_prep(inputs):
    """Host-side scalar/layout prep. Returns per-core in_maps."""
    f32 = np.float32
    f64 = np.float64
    datax = np.asarray(inputs["datax"])
    embMat = np.asarray(inputs["embMat"], f32)
    Wq = np.asarray(inputs["Wq"], f32)
    Wk = np.asarray(inputs["Wk"], f32)
    k1 = np.ascontiguousarray(np.asarray(inputs["k1"], f32))
    k2 = np.ascontiguousarray(np.asarray(inputs["k2"], f32))
    k3 = np.asarray(inputs["k3"], f32)
    sp = np.ascontiguousarray(np.asarray(inputs["shortestPath"], f32))
    cen = np.ascontiguousarray(np.asarray(inputs["centralityMat"], f32))
    pad = np.asarray(inputs["padding"], f32)
    fc1_w = np.asarray(inputs["fc1_w"], f32)

    xbin = datax != 0
    x0 = xbin.astype(f32)                          # [B,G]

    e0 = embMat[0].astype(f64)
    e1 = embMat[1].astype(f64)
    q0 = Wq.astype(f64) @ e0
    q1 = Wq.astype(f64) @ e1
    kv0 = Wk.astype(f64) @ e0
    kv1 = Wk.astype(f64) @ e1
    c00 = q0 @ kv0
    c01 = q0 @ kv1
    c10 = q1 @ kv0
    c11 = q1 @ kv1

    xg = x0.astype(f64)
    A = c00 + xg * (c10 - c00)                     # [B,G]
    Bc = (c01 - c00) + xg * ((c11 - c10) - (c01 - c00))
    eB = np.exp(Bc)

    k3lr = float(_leaky(k3)[0])
    padk = np.ascontiguousarray(pad * k3lr).astype(f32)

    rhs2 = np.empty((B, 2, G), f32)
    rhs2[:, 0, :] = 1.0
    rhs2[:, 1, :] = x0

    xbt2 = np.empty((G, 32), f32)
    xbt2[:, :B] = x0.T
    xbt2[:, B:] = 1.0 - x0.T
    xbt2p = np.ascontiguousarray(xbt2.reshape(HCH, 128, 32).transpose(1, 0, 2))

    fc1wT = np.ascontiguousarray(fc1_w.T)          # [g, j]

    def tslice(arr_bg, sl):                        # [B,Rslice] -> [128, TPC, B]
        return np.ascontiguousarray(
            arr_bg[:, sl].T.reshape(TPC, 128, B).transpose(1, 0, 2)
        ).astype(f32)

    def _ab_block(A, Bc, sl):
        # [32, B*R]: column block b carries A/Bc in rows 2b/2b+1, zeros elsewhere.
        out = np.zeros((2 * B, B * R), f32)
        for b in range(B):
            out[2 * b, b * R:(b + 1) * R] = A[b, sl]
            out[2 * b + 1, b * R:(b + 1) * R] = Bc[b, sl]
        return out

    in_maps = []
    for c in range(NCORES):
        sl = slice(c * R, (c + 1) * R)
        in_maps.append({
            "kk1": k1[sl].reshape(TPC, 128, G),
            "ksp": sp[sl].reshape(TPC, 128, G),
            "kk2": k2[sl].reshape(TPC, 128, G),
            "kcen": cen[sl].reshape(TPC, 128, G),
            "kpad": padk[sl].reshape(TPC, 128, G),
            "rhs2": rhs2,
            "ab": _ab_block(A, Bc, sl),
            "xbt2": xbt2p,
            "ebt": tslice(eB, sl),
            "xgt": tslice(xg, sl),
            "fc1t": fc1wT[sl].reshape(TPC, 128, G),
        })
    return in_maps


def kernel(**inputs):
    _ensure_path()
    from concourse.bass_utils import run_bass_kernel_spmd

    in_maps = host_prep(inputs)
    nc = build_nc()

    res = run_bass_kernel_spmd(nc, in_maps, core_ids=list(range(NCORES)))
    kernel.last_results = res
    kernel.last_exec_time_ns = res.exec_time_ns

    f32 = np.float32
    attn_w = np.empty((B, G, G), f32)
    fc1sum = np.zeros((B, G), np.float64)
    for c in range(NCORES):
        r = res.results[c]
        attn_w[:, c * R:(c + 1) * R, :] = r["wout"]
        fc1sum += r["fc1p"]

    fc1_b = np.asarray(inputs["fc1_b"], f32)
    fcCox_w = np.asarray(inputs["fcCox_w"], f32)
    y = fc1sum + fc1_b.astype(np.float64)
    out = (y @ fcCox_w[0].astype(np.float64)).astype(f32)
    return out, attn_w
